# revision 51
# baseline (speedup 1.0000x reference)
"""Local (Gaussian-banded) attention kernel for Trainium2, 8 NeuronCores.

Math: out = rownorm(gauss_band(sigma)) @ (x @ Wg) @ Wout
The Gaussian positional mask with sigma in [0.5, 2.5] has < 2e-6 relative
tail mass past |i-j| > 12, so attention is a 25-tap banded matmul.

Sharding: core c = (batch b = c//2, seq-half s = c%2). Each core computes
out rows [s*1024, (s+1)*1024) of its batch. s=1 halves are row-reversed on
host so the sequence edge is always at local row 0 -> all 8 cores run the
same program with the same band constants (pure SPMD).

v15 (~57us, from 64-72us v4): chunk-interleaved single-warm-window
schedule. What mattered, in order:
 1. Interleave the band stage's short-stream, LDW-per-matmul units
    between long N=512 streams of stages 1/3 (single-matmul granularity)
    so the PE weight-load scoreboard drains under real work; v4 paced
    the band stage at ~(mm+ldw+drain)/2 AND its low duty cycle HAM-
    re-throttled the PE to 1.2 GHz for the following output GEMM.
 2. All input DMAs on ONE HWDGE ring (sync), ordered by first use
    [wgA | xT t0 | wgB | xT t1-2 | xT t3-8 | bands | wout]: concurrent
    DMAs fair-share the 16 SDMA engines at packet granularity, so a
    small early transfer's completion sem otherwise straggles ~5us
    behind later traffic. Only the ring-head DMA completes ~0.4us after
    last byte; later ones lag +1.5-2us.
 3. N=512 garbage warmups (reading uninitialized SBUF) bridge the
    preamble->wgA-arrival window (~7->13us) to hold the HAM activity
    window; tile 0 runs psA x4 then psB x4 unpaired so compute starts
    on wgA alone while wgB is in flight.
 4. Band tables deduped to one Toeplitz piece B [128, 128+2W] per head
    (chunk-leading A = B[:,2W:], corner C = B[:2W,:2W]) + W edge-
    renormalized columns A0e; W=12.
 5. PSUM->SBUF casts alternate Vector/Scalar engines; stage-1 psA/psB
    pairs share one LDWEIGHTS via post-compile strip; outputs ride the
    (by then idle) sync HWDGE ring, final 512 cols split 2x256 so the
    last DMA chases a short cast.
Stage structure:
  stage 1: v[1048,1024] = xpad @ Wg, tile-major xT layout.
  stage 2: per (head, 512-col chunk q): A|B|B|B|C band pieces, interior
           row-norm baked in; q0's first W cols use edge-renormed A0e.
  stage 3: out = attn chunk @ Wout (8 accumulating N=512 matmuls per
           128-row block).
Emission order: warmup; s1 t0-t4; [s2(h,q0) x8 interleaved with s1
t5-t8]; [s2(h,q1) x8 interleaved with s3 blocks i0,i1]; s3 i2,i3.
Remaining time: ~7us framework preamble, ~5us DMA latency to first
weights, ~33us PE stream (bf16 roofline for this decomposition), ~3us
cast+DMA tail, ~3us counted epilogue. fp8 would halve the stream but
e4m3's ~5% dot-product error exceeds the 2e-2 budget.
Measured rel err vs fp32 reference ~4.5e-3.
"""

import sys

for _p in ("/opt/trn_rl_repo", "/root/.axon_site/_ro/trn_rl_repo"):
    if _p not in sys.path:
        sys.path.append(_p)

import numpy as np
import ml_dtypes

BF16 = ml_dtypes.bfloat16

B, N, D = 4, 2048, 512
H, DH = 8, 128
INNER = H * DH
W = 12                      # band half-width (tail mass < 2e-6 at sigma=2.5)
PAD = W                     # zero pad rows
HALO = W
TROWS = 2 * W               # rows of the last partial v tile
VROWS = PAD + 1024 + HALO   # 1048 = 8*128 + 24
NT = 9                      # v tiles: 8 x 128 + 1 x TROWS
XCOLS = 8 * 512 + 4 * TROWS  # tile-major xT: t<8 at t*512+k*128, t8 at 4096+k*TROWS
BW = 128 + 2 * W            # generic band piece width
STRIP_ON = True             # strip LDW on stage-1 psB (reuses psA stationary)
NWARM = 12                  # N=512 PE warmup matmuls during the input DMA wait
NWARM_S = 0                 # short N=128 warmup fillers bridging the wg wait
# consts column map (bf16): A0e | B | wout. The generic interior band
# piece B [128,BW] covers everything Toeplitz: the chunk-leading A
# piece is B[:,2W:2W+128] (same offsets shifted), the corner C piece is
# B[:2W,:2W]; only the W edge-renormalized columns need their own
# A0e [128,W] per head.
CA0 = 0
CB = H * W
CW = CB + H * BW
CTOT = CW + H * 512

_CACHE = {}


def _build_nc():
    import concourse.mybir as mybir
    from concourse import bacc
    from concourse.tile import TileContext

    f32 = mybir.dt.float32
    bf = mybir.dt.bfloat16
    act_copy = mybir.ActivationFunctionType.Copy

    nc = bacc.Bacc(None, target_bir_lowering=False)
    strip = []

    def mm(out, lhsT, rhs, reuse=False, **kw):
        i = nc.tensor.matmul(out, lhsT, rhs, **kw)
        if reuse and STRIP_ON:
            strip.append(i.ins.name)
        return i

    xT = nc.dram_tensor("xT", [128, XCOLS], bf, kind="ExternalInput")
    wg = nc.dram_tensor("Wg2", [128, 8 * 512], bf, kind="ExternalInput")
    consts = nc.dram_tensor("consts", [128, CTOT], bf, kind="ExternalInput")
    # out rows r = i*256 + half*128 + p  ->  cols i*1024 + half*512 + d
    out = nc.dram_tensor("out", [128, 4 * 1024], bf, kind="ExternalOutput")

    with TileContext(nc) as tc:
        with (
            tc.tile_pool(name="persist", bufs=1) as pp,
            tc.tile_pool(name="outs", bufs=4) as osp,
            tc.tile_pool(name="ps1", bufs=2, space="PSUM") as ps1,
            tc.tile_pool(name="ps2", bufs=2, space="PSUM") as ps2,
            tc.tile_pool(name="ps3", bufs=2, space="PSUM") as ps3,
        ):
            xT_sb = pp.tile([128, XCOLS], bf, tag="xT", name="xT_sb")
            wg_sb = pp.tile([128, 8 * 512], bf, tag="wg", name="wg_sb")
            wgA_sb, wgB_sb = wg_sb[:, :2048], wg_sb[:, 2048:]
            cs = pp.tile([128, CTOT], bf, tag="consts", name="cs")
            scratch = pp.tile([128, 1], bf, tag="scr", name="scr")
            # Early inputs ride ONE HWDGE ring (sync) in first-use
            # order: concurrent DMAs fair-share the 16 SDMA engines at
            # packet granularity, so a transfer's completion (the
            # slowest engine's last packet) straggles behind every
            # later concurrent DMA; and each ring processes its DMAs
            # serially with ~0.7us per-DMA overhead, so pieces are few
            # and large. Only the ring-head DMA's completion sem fires
            # ~0.4us after its last byte (later ones lag +1.5-2us), so
            # wgA — the first thing compute needs — goes first.
            import os as _os
            _XT1_SCALAR = _os.environ.get("K_XT1_SCALAR", "0") == "1"
            nc.sync.dma_start(out=wg_sb[:, :2048], in_=wg[:, :2048])
            if _XT1_SCALAR:
                # both startup-critical pieces become ring-heads
                nc.scalar.dma_start(out=xT_sb[:, :512], in_=xT[:, :512])
            else:
                nc.sync.dma_start(out=xT_sb[:, :512], in_=xT[:, :512])
            nc.sync.dma_start(out=wg_sb[:, 2048:], in_=wg[:, 2048:])
            nc.sync.dma_start(out=xT_sb[:, 512:1536], in_=xT[:, 512:1536])
            nc.sync.dma_start(out=xT_sb[:, 1536:], in_=xT[:, 1536:])
            nc.sync.dma_start(out=cs[:, :CW], in_=consts[:, :CW])
            nc.sync.dma_start(out=cs[:, CW:], in_=consts[:, CW:])

            v_sb = [pp.tile([128, INNER], bf, tag=f"v{t}", name=f"v{t}")
                    for t in range(NT)]
            # attnT: [dh=128, (h, out col 0..1024)] bf16
            attnT = pp.tile([128, H * 1024], bf, tag="attnT", name="attnT")

            # PE warmup while inputs stream in: the HAM clock gate needs
            # ~3.4us of sustained matmul activity to lift the PE from 1.2
            # to 2.4 GHz. Reads uninitialized attnT (values irrelevant,
            # results discarded) so it has no upstream dependency at all.
            # A tail of short N=128 fillers bridges the remaining wg-DMA
            # wait at fine granularity so a late arrival doesn't idle the
            # PE into a HAM re-throttle and an early one wastes little.
            for w in range(NWARM):
                pw = ps3.tile([128, 512], f32, tag="s3", name=f"warm{w}")
                nc.tensor.matmul(pw, attnT[:, 0:128], attnT[:, 128:640],
                                 start=True, stop=True)
            for w in range(NWARM_S):
                pw = ps3.tile([128, 512], f32, tag="s3", name=f"warms{w}")
                nc.tensor.matmul(pw[:, :128], attnT[:, 0:128],
                                 attnT[:, 128:256], start=True, stop=True)

            ncast = [0]

            def cast(dst, src):
                # alternate PSUM->SBUF casts between DVE and ACT so
                # neither engine's copy queue gates the PE
                if ncast[0] % 2 == 0:
                    nc.vector.tensor_copy(dst, src)
                else:
                    nc.scalar.activation(dst, src, act_copy)
                ncast[0] += 1

            def s1_thunks(t):
                # 4 weight-sharing pair thunks + a cast thunk; pairs may
                # be interleaved with other-bank matmuls, each pair stays
                # atomic so the LDW strip keeps its adjacency
                rows = 128 if t < 8 else TROWS
                xoff = t * 512 if t < 8 else 4096
                kstep = 128 if t < 8 else TROWS
                st = {}

                def pair(k):
                    if k == 0:
                        # one 2-bank tile so the A/B halves allocate
                        # atomically
                        st["ps"] = ps1.tile([128, 1024], f32, tag="s1",
                                            name=f"psT{t}")
                    psT = st["ps"]
                    lh = xT_sb[:, xoff + k * kstep: xoff + k * kstep + rows]
                    mm(psT[:rows, 0:512], lh,
                       wgA_sb[:, k * 512:(k + 1) * 512],
                       start=(k == 0), stop=(k == 3))
                    mm(psT[:rows, 512:1024], lh,
                       wgB_sb[:, k * 512:(k + 1) * 512],
                       reuse=True, start=(k == 0), stop=(k == 3))

                def half(k, col):
                    # unpaired: psA sweep then psB sweep (own LDWs) so
                    # tile 0's psA matmuls can start on wgA alone while
                    # wgB is still in flight
                    if k == 0 and col == 0:
                        st["ps"] = ps1.tile([128, 1024], f32, tag="s1",
                                            name=f"psT{t}")
                    wsb = wgA_sb if col == 0 else wgB_sb
                    mm(st["ps"][:rows, col * 512:col * 512 + 512],
                       xT_sb[:, xoff + k * kstep: xoff + k * kstep + rows],
                       wsb[:, k * 512:(k + 1) * 512],
                       start=(k == 0), stop=(k == 3))

                def fin():
                    cast(v_sb[t][:rows, :], st["ps"][:rows, :])

                if t == 0:
                    return ([lambda k=k: half(k, 0) for k in range(4)]
                            + [lambda k=k: half(k, 1) for k in range(4)]
                            + [fin])
                return [lambda k=k: pair(k) for k in range(4)] + [fin]

            def s2_thunks(h, q):
                # band stage for one head, one 512-col chunk: all pieces
                # are views of the shared Toeplitz B; chunk q0's first 16
                # cols use the edge-renormalized A0e piece
                hs = slice(h * 128, (h + 1) * 128)
                bh = CB + h * BW
                st = {}

                def t_a():
                    st["ps"] = ps2.tile([128, 512], f32, tag="s2",
                                        name=f"s2_{h}_{q}")
                    P = st["ps"]
                    if q == 0:
                        mm(P[:, 0:W], v_sb[0][:, hs],
                           cs[:, CA0 + h * W: CA0 + (h + 1) * W],
                           start=True, stop=False)
                        mm(P[:, W:128], v_sb[0][:, hs],
                           cs[:, bh + 3 * W: bh + BW],
                           reuse=True, start=False, stop=False)
                    else:
                        mm(P[:, 0:128], v_sb[4][:, hs],
                           cs[:, bh + 2 * W: bh + BW],
                           start=True, stop=False)

                def t_b(j):
                    mm(st["ps"][:, j * 128 - 2 * W: j * 128 + 128],
                       v_sb[4 * q + j][:, hs], cs[:, bh: bh + BW],
                       start=False, stop=False)

                def t_c():
                    mm(st["ps"][:, 512 - 2 * W:512],
                       v_sb[4 * q + 4][:2 * W, hs],
                       cs[:2 * W, bh: bh + 2 * W], start=False, stop=True)
                    cast(attnT[:, h * 1024 + q * 512:
                               h * 1024 + q * 512 + 512], st["ps"])

                return [t_a] + [lambda j=j: t_b(j) for j in (1, 2, 3)] + [t_c]

            ots = {}
            psmap = {}

            def s3_mm(i, half, h):
                if h == 0:
                    psmap[(i, half)] = ps3.tile([128, 512], f32, tag="s3",
                                                name=f"ps3_{i}_{half}")
                ps = psmap[(i, half)]
                off = h * 1024 + i * 256 + half * 128
                mm(ps, attnT[:, off:off + 128],
                   cs[:, CW + h * 512: CW + (h + 1) * 512],
                   start=(h == 0), stop=(h == 7))
                if h == 7:
                    s3_close(ps, i, half)

            def s3_close(ps, i, half):
                if i not in ots:
                    ots[i] = osp.tile([128, 1024], bf, tag="outt", name=f"ot{i}")
                ot = ots[i]
                # output DMAs ride the sync HWDGE ring (idle after the
                # input phase; ~0.4us lower first-byte latency than SWDGE)
                if i == 3 and half == 1:
                    # the very last piece: cast+DMA in 2x256-col slices
                    # so the first DMA's transfer hides the second cast
                    for piece in range(2):
                        sl = slice(512 + piece * 256, 512 + piece * 256 + 256)
                        cast(ot[:, sl], ps[:, piece * 256:piece * 256 + 256])
                        nc.sync.dma_start(out=out[:, 3 * 1024 + 512 + piece * 256:
                                                  3 * 1024 + 512 + piece * 256 + 256],
                                          in_=ot[:, sl])
                    return
                cast(ot[:, half * 512:(half + 1) * 512], ps)
                if i == 3:
                    # final chunk: DMA per half so the first half's
                    # transfer hides under the second half's matmuls
                    nc.sync.dma_start(
                        out=out[:, i * 1024 + half * 512:
                                i * 1024 + (half + 1) * 512],
                        in_=ot[:, half * 512:(half + 1) * 512])
                elif half == 1:
                    nc.sync.dma_start(out=out[:, i * 1024:(i + 1) * 1024], in_=ot)

            # ---- emission order (drives the Tile scheduler's priorities)
            # stage-2's short-stream, LDW-per-matmul units are interleaved
            # at single-matmul granularity between long N=512 streams of
            # stages 1/3 so the PE weight-load scoreboard drains under
            # real work and the HAM activity window stays full
            for t in range(5):
                for th in s1_thunks(t):
                    th()

            s1q = [th for t in range(5, 9) for th in s1_thunks(t)]
            slot = 0
            for h in range(H):
                for th in s2_thunks(h, 0):
                    th()
                    if slot % 2 == 0 and s1q:
                        s1q.pop(0)()
                    slot += 1
            while s1q:
                s1q.pop(0)()
            s3q = [(i, half, hh) for i in (0, 1) for half in (0, 1)
                   for hh in range(H)]
            slot = 0
            for h in range(H):
                for th in s2_thunks(h, 1):
                    th()
                    if s3q and (slot % 5) != 4:
                        s3_mm(*s3q.pop(0))
                    slot += 1
            while s3q:
                s3_mm(*s3q.pop(0))
            for i in (2, 3):
                for half in range(2):
                    for hh in range(H):
                        s3_mm(i, half, hh)

    # Strip redundant InstLdweights: simulate the tensor queue in final
    # block order tracking the loaded stationary; a marked matmul's own
    # LDW is deleted only when the currently-loaded weights already cover
    # it (same tensor/offset/cols, partition-count superset). The Tile
    # scheduler may reorder pairs, so coverage is checked, not assumed.
    import concourse.mybir as mybir
    names = set(strip)

    def sig(ap):
        p = list(ap.ap)
        return (ap.memref, ap.offset, tuple(p[1]), p[0][1], p[0][0])

    def covers(loaded, w):
        return (loaded is not None and loaded[0] == w[0] and loaded[1] == w[1]
                and loaded[2] == w[2] and loaded[4] == w[4]
                and w[3] <= loaded[3])

    removed = 0
    kept = 0
    for blk in nc.m.functions[0].blocks:
        insts = blk.instructions
        loaded = None
        pend = None          # (idx, sig, no_waits) of LDW awaiting its matmul
        dels = []
        for idx in range(len(insts)):
            inst = insts[idx]
            if isinstance(inst, mybir.InstLdweights):
                si = inst.sync_info
                assert pend is None, "two LDWs with no matmul between"
                pend = (idx, sig(inst.ins[0]),
                        si is None or len(si.on_wait) == 0)
            elif isinstance(inst, mybir.InstMatmult):
                w = sig(inst.ins[1])
                if pend is not None:
                    assert pend[1] == w, (pend[1], w)
                    if inst.name in names and covers(loaded, w) and pend[2]:
                        dels.append(pend[0])
                        removed += 1
                    else:
                        loaded = pend[1]
                        if inst.name in names:
                            kept += 1
                    pend = None
                else:
                    assert covers(loaded, w), (loaded, w)
        for idx in reversed(dels):
            del insts[idx]
    if removed + kept:
        sys.stderr.write(f"ldw strip: removed {removed}, kept {kept}\n")
    nc.compile()
    return nc


def _band_constants(sigma: np.ndarray):
    """Toeplitz band pieces (interior row-norm baked in), float64 host.

    Only two tables per head: the generic interior piece B [128,160]
    (weight(i,j) = wts[|i-j+16|]/s_int; its col-slices [32:160] and
    row/col corner [:32,:32] serve as the chunk-leading A and corner C
    pieces) and A0e [128,16], the first 16 output columns with the
    sequence-edge renormalization baked in.
    """
    sig = np.asarray(sigma, np.float64).reshape(H)
    d = np.arange(W + 1, dtype=np.float64)
    wts = np.exp(-(d[None, :] ** 2) / (2.0 * sig[:, None] ** 2))  # [H, 17]
    tail = wts[:, 1:].sum(1)
    s_int = wts[:, 0] + 2.0 * tail

    i = np.arange(128)[:, None]
    j = np.arange(BW)[None, :]
    o = np.abs(i - j + W)
    Bp = np.where((o <= W)[None], wts[:, np.minimum(o, W).astype(int)], 0.0)
    Bp = Bp / s_int[:, None, None]                        # [H, 128, BW]
    # edge renorm for out cols 0..W-1 (left-truncated gaussian)
    je = np.arange(W)
    cum = np.concatenate([np.zeros((H, 1)), np.cumsum(wts[:, 1:], 1)], 1)
    s_edge = wts[:, [0]] + cum[:, je] + tail[:, None]     # [H, W]
    A0e = Bp[:, :, 2 * W:3 * W] * (s_int[:, None] / s_edge)[:, None, :]

    def pack(x):
        # [H, ni, nj] -> [ni, H*nj]
        return np.ascontiguousarray(x.transpose(1, 0, 2).reshape(
            x.shape[1], H * x.shape[2])).astype(BF16)

    return pack(A0e), pack(Bp)


def _pack_k(a, cols):
    # [512, cols] -> [128, 4*cols] with partition p = d%128, k = d//128
    return np.ascontiguousarray(
        a.reshape(4, 128, cols).transpose(1, 0, 2).reshape(128, 4 * cols))


def _consts(Wout, sigma):
    a0e, b = _band_constants(sigma)
    cs = np.zeros((128, CTOT), BF16)
    cs[:, CA0:CB] = a0e
    cs[:, CB:CW] = b
    cs[:, CW:] = np.asarray(Wout, BF16).reshape(H, 128, D).transpose(1, 0, 2) \
        .reshape(128, H * D)
    return cs


def _in_maps(x, Wg, Wout, sigma):
    cs = _consts(Wout, sigma)
    wgf = np.asarray(Wg, BF16)
    wg2 = np.concatenate([
        _pack_k(np.ascontiguousarray(wgf[:, 0:512]), 512),
        _pack_k(np.ascontiguousarray(wgf[:, 512:1024]), 512)], axis=1)
    wg2 = np.ascontiguousarray(wg2)
    x = np.asarray(x, np.float32)
    maps = []
    for ci in range(8):
        b, s = divmod(ci, 2)
        z = x[b] if s == 0 else x[b, ::-1]
        xbuf = np.zeros((VROWS, D), np.float32)
        xbuf[PAD:] = z[:1024 + HALO]
        xt = np.ascontiguousarray(xbuf.T).astype(BF16)   # [512, 1056]
        xtp = np.zeros((128, XCOLS), BF16)
        for k in range(4):
            blk = xt[k * 128:(k + 1) * 128]              # [128, 1056]
            for t in range(8):
                xtp[:, t * 512 + k * 128: t * 512 + (k + 1) * 128] = \
                    blk[:, t * 128:(t + 1) * 128]
            xtp[:, 4096 + k * TROWS: 4096 + (k + 1) * TROWS] = blk[:, 1024:]
        maps.append({"xT": xtp, "Wg2": wg2, "consts": cs})
    return maps


def _get_nc():
    if "nc" not in _CACHE:
        _CACHE["nc"] = _build_nc()
    return _CACHE["nc"]


def run_spmd(in_maps, **kw):
    from concourse.bass_utils import run_bass_kernel_spmd
    return run_bass_kernel_spmd(_get_nc(), in_maps, core_ids=list(range(8)), **kw)


def _assemble(results):
    full = np.empty((B, N, D), np.float32)
    for c in range(8):
        b, s = divmod(c, 2)
        r = results[c]["out"]          # [128, 4096] bf16
        r = r.astype(np.float32).reshape(128, 4, 2, 512)
        r = r.transpose(1, 2, 0, 3).reshape(1024, 512)
        if s == 0:
            full[b, :1024] = r
        else:
            full[b, 1024:] = r[::-1]
    return full


def kernel(x, Wg, Wout, sigma):
    res = run_spmd(_in_maps(x, Wg, Wout, sigma))
    return _assemble(res.results)
